# revision 4
# baseline (speedup 1.0000x reference)
"""Batched brute-force k-NN (k=16) on 8 Trainium2 NeuronCores.

Problem: ref [4, 8192, 3] f32, query [4, 4096, 3] f32 ->
         dist [4, 4096, 16] f32, idx [4, 4096, 16] int32 (top-16 smallest
         Euclidean distances per query, ascending).

Sharding: 8 cores = 4 batches x 2 query-halves. Each core handles one
batch's full ref set (8192 refs) and 2048 queries.

Per-core algorithm:
  score[q, r] = 2 q.r - ||r||^2   (= -||q-r||^2 + ||q||^2, same per-query
  ranking since ||q||^2 is constant along a query row)
  via a K=6 matmul per (128-query, 512-ref) tile:
    lhsT rows: [qx, qy, qz, -1, -1, -1]
    rhs  rows: [2rx, 2ry, 2rz, rx^2, ry^2, rz^2]
  Top-16 per query row = DVE max8 -> max_index -> match_replace(-inf)
  -> max8 -> max_index (exact fp32 values; ties resolved in index order,
  matching jax.lax.top_k). Then dist = sqrt(relu(||q||^2 - score)) with
  ||q||^2 folded in as the activation's per-partition bias.
"""

import sys

sys.path.insert(0, "/opt/trn_rl_repo")

import numpy as np

B, NR, NQ, D, K = 4, 8192, 4096, 3, 16
N_CORES = 8
QPC = NQ // 2  # queries per core: 2048
NEG_INF = -3.0e38

_CACHE = {}


def _build_nc(nq=QPC, nr=NR):
    import concourse.bacc as bacc
    import concourse.mybir as mybir
    import concourse.tile as tile

    f32 = mybir.dt.float32
    AF = mybir.ActivationFunctionType

    n_qt = nq // 128  # query tiles
    n_rt = nr // 512  # ref chunks per row

    nc = bacc.Bacc(
        "TRN2", target_bir_lowering=False, debug=False, num_devices=N_CORES
    )
    ref_d = nc.dram_tensor("ref", [nr, D], f32, kind="ExternalInput")
    q_d = nc.dram_tensor("query", [nq, D], f32, kind="ExternalInput")
    dist_d = nc.dram_tensor("dist", [nq, K], f32, kind="ExternalOutput")
    idx_d = nc.dram_tensor("idx", [nq, K], mybir.dt.int32, kind="ExternalOutput")

    with tile.TileContext(nc) as tc:
        with tc.tile_pool(name="const", bufs=1) as cpool, tc.tile_pool(
            name="rows", bufs=2
        ) as rpool, tc.tile_pool(name="small", bufs=3) as spool, tc.tile_pool(
            name="psum", bufs=8, space="PSUM"
        ) as ppool:
            refT = cpool.tile([D, nr], f32)
            nc.sync.dma_start(out=refT[:, :], in_=ref_d.ap().transpose([1, 0]))

            # rhs rows: [2r (0:3), r^2 (3:6)]. Engine writes must start at a
            # 32-aligned partition, so r^2 goes through an aligned scratch
            # tile and an SBUF->SBUF DMA (DMAs have no alignment rule).
            rhs = cpool.tile([2 * D, nr], f32)
            rsq = cpool.tile([D, nr], f32)
            nc.scalar.activation(out=rhs[0:D, :], in_=refT[:, :], func=AF.Copy, scale=2.0)
            nc.scalar.activation(out=rsq[:, :], in_=refT[:, :], func=AF.Square)
            nc.sync.dma_start(out=rhs[D : 2 * D, :], in_=rsq[:, :])

            # lhsT rows: [q (0:3), -1 (3:6)]: memset all to -1, then DMA the
            # transposed query block over rows 0:3.
            lhsT = cpool.tile([2 * D, nq], f32)
            nc.vector.memset(lhsT[:, :], -1.0)
            nc.sync.dma_start(out=lhsT[0:D, :], in_=q_d.ap().transpose([1, 0]))

            # ||q||^2 per query, laid out [128, n_qt]: natural-layout load +
            # ACT Square with free-axis accumulation.
            qnat = cpool.tile([128, n_qt, D], f32)
            nc.sync.dma_start(
                out=qnat[:, :, :],
                in_=q_d.ap().rearrange("(t p) d -> p t d", p=128),
            )
            qn2 = cpool.tile([128, n_qt], f32)
            qsq = cpool.tile([128, n_qt, D], f32)
            for qt in range(n_qt):
                nc.scalar.activation(
                    out=qsq[:, qt, :],
                    in_=qnat[:, qt, :],
                    func=AF.Square,
                    accum_out=qn2[:, qt : qt + 1],
                )

            for qt in range(n_qt):
                row = rpool.tile([128, nr], f32)
                for j in range(n_rt):
                    ps = ppool.tile([128, 512], f32)
                    nc.tensor.matmul(
                        ps[:, :],
                        lhsT[:, qt * 128 : (qt + 1) * 128],
                        rhs[:, j * 512 : (j + 1) * 512],
                        start=True,
                        stop=True,
                    )
                    nc.scalar.copy(out=row[:, j * 512 : (j + 1) * 512], in_=ps[:, :])

                scores = spool.tile([128, K], f32, tag="scores")
                idxs = spool.tile([128, K], mybir.dt.uint32, tag="idxs")
                nc.vector.max(out=scores[:, 0:8], in_=row[:, :])
                nc.vector.max_index(
                    out=idxs[:, 0:8], in_max=scores[:, 0:8], in_values=row[:, :]
                )
                nc.vector.match_replace(
                    out=row[:, :],
                    in_to_replace=scores[:, 0:8],
                    in_values=row[:, :],
                    imm_value=NEG_INF,
                )
                nc.vector.max(out=scores[:, 8:16], in_=row[:, :])
                nc.vector.max_index(
                    out=idxs[:, 8:16], in_max=scores[:, 8:16], in_values=row[:, :]
                )

                # sq_dist = relu(||q||^2 - score); dist = sqrt(sq_dist)
                sq = spool.tile([128, K], f32, tag="sq")
                dist = spool.tile([128, K], f32, tag="dist")
                nc.scalar.activation(
                    out=sq[:, :],
                    in_=scores[:, :],
                    func=AF.Relu,
                    scale=-1.0,
                    bias=qn2[:, qt : qt + 1],
                )
                nc.scalar.activation(out=dist[:, :], in_=sq[:, :], func=AF.Sqrt)

                qs = qt * 128
                nc.sync.dma_start(out=dist_d.ap()[qs : qs + 128, :], in_=dist[:, :])
                nc.sync.dma_start(
                    out=idx_d.ap()[qs : qs + 128, :],
                    in_=idxs[:, :].bitcast(mybir.dt.int32),
                )

    nc.finalize()
    return nc


def kernel(ref: np.ndarray, query: np.ndarray):
    from concourse.bass_utils import run_bass_kernel_spmd

    if "nc" not in _CACHE:
        _CACHE["nc"] = _build_nc()
    nc = _CACHE["nc"]

    ref = np.asarray(ref, dtype=np.float32)
    query = np.asarray(query, dtype=np.float32)

    in_maps = []
    for c in range(N_CORES):
        b, h = c // 2, c % 2
        in_maps.append(
            {
                "ref": np.ascontiguousarray(ref[b]),
                "query": np.ascontiguousarray(query[b, h * QPC : (h + 1) * QPC]),
            }
        )

    res = run_bass_kernel_spmd(nc, in_maps, list(range(N_CORES)))
    _CACHE["last_res"] = res

    dist = np.empty((B, NQ, K), dtype=np.float32)
    idx = np.empty((B, NQ, K), dtype=np.int32)
    for c in range(N_CORES):
        b, h = c // 2, c % 2
        dist[b, h * QPC : (h + 1) * QPC] = res.results[c]["dist"]
        idx[b, h * QPC : (h + 1) * QPC] = res.results[c]["idx"].astype(np.int32)
    return dist, idx
